# revision 1
# baseline (speedup 1.0000x reference)
"""MoE layer kernel for Trainium2 (8 NeuronCores, Bass/Tile).

Strategy: the reference runs all E=8 experts densely on all B=8192 tokens,
but only the top-2 experts per token contribute to the output. We compute
the (cheap) gate + top-2 routing on the host, then run only the routed
(token, expert) pairs on the device (4x less FLOPs than dense).

Device sharding: (expert, token-chunk) pieces are load-balanced across the
8 cores. Each core runs the same SPMD program with S weight "slots" of
static capacities (the profile is uniform across cores; computed at runtime
from the actual routing counts, then compiled in). Slot s of core c gets
one expert's weights and a chunk of that expert's tokens.

Math on device (per slot): h1 = relu(x@W1+b1); h2 = relu(h1@W2+b2);
out = h2@W3+b3, computed feature-major (features on partitions, tokens on
the free axis) in bf16 with fp32 PSUM accumulation.
"""

import numpy as np
import ml_dtypes

import concourse.bacc as bacc
import concourse.mybir as mybir
from concourse import tile
from concourse.bass_utils import run_bass_kernel_spmd

BF16 = ml_dtypes.bfloat16

D, H, H2, DOUT, E, B, TOP_K = 1024, 4096, 2048, 1024, 8, 8192, 2
NCORES = 8
CAP_MAX = 1280  # max slot capacity s.t. activations stay resident in SBUF
QUANT = 128

_program_cache: dict = {}


# ---------------------------------------------------------------- routing

def _gate_and_route(x, Wg1, bg1, Wg2, bg2):
    g = np.maximum(x @ Wg1 + bg1, 0.0)
    logits = g @ Wg2 + bg2
    m = logits.max(axis=-1, keepdims=True)
    e = np.exp(logits - m)
    gate_w = e / e.sum(axis=-1, keepdims=True)  # [B, E]

    idx = np.argpartition(-gate_w, TOP_K - 1, axis=1)[:, :TOP_K]
    vals = np.take_along_axis(gate_w, idx, 1)
    order = np.argsort(-vals, axis=1, kind="stable")
    top_idx = np.take_along_axis(idx, order, 1)  # [B, 2] descending
    top_v = np.take_along_axis(vals, order, 1)
    # renormalize with softmax over the selected gate *values*
    tm = top_v.max(axis=-1, keepdims=True)
    te = np.exp(top_v - tm)
    top_w = te / te.sum(axis=-1, keepdims=True)  # [B, 2]
    return gate_w, top_idx, top_w


# ------------------------------------------------------- slot-profile plan

def _feasible(profile, demands):
    """Greedy cover of per-expert demands with 8 instances of each slot size.

    Returns per-expert list of slot sizes used, or None."""
    pool = {}
    for c in profile:
        pool[c] = pool.get(c, 0) + NCORES
    sizes = sorted(pool, reverse=True)
    take = {e: [] for e in range(len(demands))}
    for e in np.argsort(-np.asarray(demands)):
        rem = demands[e]
        while rem > 0:
            avail = [c for c in sizes if pool[c] > 0]
            if not avail:
                return None
            if rem >= avail[0]:
                c = avail[0]
            else:
                c = min(c for c in avail if c >= rem)
            pool[c] -= 1
            take[e].append(c)
            rem -= c
    return take


def plan_slots(counts):
    """Pick a slot-capacity profile (uniform across cores) and assign
    (expert, piece) -> (core, slot)."""
    demands = [int(c) for c in counts]
    total = sum(demands)
    best = None
    for c1 in range(CAP_MAX, 0, -QUANT):
        for c2 in range(c1, -1, -QUANT):
            for c3 in range(c2, -1, -QUANT):
                for c4 in [0] if c3 == 0 else range(c3, -1, -QUANT):
                    prof = tuple(c for c in (c1, c2, c3, c4) if c > 0)
                    if NCORES * sum(prof) < total:
                        continue
                    cost = NCORES * sum(prof) + 500 * len(prof)
                    if best is not None and cost >= best[0]:
                        continue
                    take = _feasible(prof, demands)
                    if take is not None:
                        best = (cost, prof, take)
    assert best is not None, "no feasible slot profile"
    _, prof, take = best

    # map instances to (core, slot_pos): slot position s has 8 instances
    # (cores 0..7); duplicate sizes in prof occupy distinct positions.
    pos_of_size = {}
    for s, c in enumerate(prof):
        pos_of_size.setdefault(c, []).append(s)
    free = {c: [(core, s) for s in pos_of_size[c] for core in range(NCORES)]
            for c in pos_of_size}
    # assignment[(core, slot)] = (expert, tok_offset_in_expert_list, length)
    assignment = {}
    for e, sizes_taken in take.items():
        off = 0
        rem = int(counts[e])
        for c in sorted(sizes_taken, reverse=True):
            core, s = free[c].pop()
            ln = min(rem, c)
            assignment[(core, s)] = (e, off, ln)
            off += ln
            rem -= ln
    return prof, assignment


# ------------------------------------------------------- device program

def _build_program(profile):
    nc = bacc.Bacc("TRN2", target_bir_lowering=False, debug=False)
    S = len(profile)
    d_in, d_w1, d_b1, d_w2, d_b2, d_w3, d_b3, d_out = [], [], [], [], [], [], [], []
    for s, C in enumerate(profile):
        d_in.append(nc.dram_tensor(f"xT{s}", (8, 128, C), mybir.dt.bfloat16,
                                   kind="ExternalInput"))
        d_w1.append(nc.dram_tensor(f"w1_{s}", (128, 32, 8, 128), mybir.dt.bfloat16,
                                   kind="ExternalInput"))
        d_b1.append(nc.dram_tensor(f"b1_{s}", (128, 32), mybir.dt.float32,
                                   kind="ExternalInput"))
        d_w2.append(nc.dram_tensor(f"w2_{s}", (128, 16, 32, 128), mybir.dt.bfloat16,
                                   kind="ExternalInput"))
        d_b2.append(nc.dram_tensor(f"b2_{s}", (128, 16), mybir.dt.float32,
                                   kind="ExternalInput"))
        d_w3.append(nc.dram_tensor(f"w3_{s}", (128, 8, 16, 128), mybir.dt.bfloat16,
                                   kind="ExternalInput"))
        d_b3.append(nc.dram_tensor(f"b3_{s}", (128, 8), mybir.dt.float32,
                                   kind="ExternalInput"))
        d_out.append(nc.dram_tensor(f"outT{s}", (8, 128, C), mybir.dt.float32,
                                    kind="ExternalOutput"))

    def nblocks(C):
        blocks, off = [], 0
        while off < C:
            sz = min(512, C - off)
            blocks.append((off, sz))
            off += sz
        return blocks

    with tile.TileContext(nc) as tc:
        with (
            tc.tile_pool(name="acts", bufs=1) as acts,
            tc.tile_pool(name="wpool", bufs=3) as wpool,
            tc.tile_pool(name="bias", bufs=2) as bias,
            tc.tile_pool(name="outp", bufs=4) as outp,
            tc.tile_pool(name="psum", bufs=6, space="PSUM") as psum,
        ):
            for s, C in enumerate(profile):
                blks = nblocks(C)
                x_sb = acts.tile([128, 8, C], mybir.dt.bfloat16, tag="x")
                h1_sb = acts.tile([128, 32, C], mybir.dt.bfloat16, tag="h1")
                h2_sb = acts.tile([128, 16, C], mybir.dt.bfloat16, tag="h2")
                b1t = bias.tile([128, 32], mybir.dt.float32, tag="b1")
                b2t = bias.tile([128, 16], mybir.dt.float32, tag="b2")
                b3t = bias.tile([128, 8], mybir.dt.float32, tag="b3")
                nc.sync.dma_start(b1t[:], d_b1[s].ap()[:])
                nc.sync.dma_start(b2t[:], d_b2[s].ap()[:])
                nc.sync.dma_start(b3t[:], d_b3[s].ap()[:])
                for ko in range(8):
                    nc.sync.dma_start(x_sb[:, ko, :], d_in[s].ap()[ko, :, :])

                # L1: h1[mo] = relu(sum_ko W1[ko,mo].T @ x[ko] + b1[mo])
                for mo in range(32):
                    w1t = wpool.tile([128, 8, 128], mybir.dt.bfloat16, tag="w1")
                    nc.sync.dma_start(w1t[:], d_w1[s].ap()[:, mo, :, :])
                    for off, sz in blks:
                        ps = psum.tile([128, 512], mybir.dt.float32, tag="ps")
                        for ko in range(8):
                            nc.tensor.matmul(
                                ps[:, :sz], w1t[:, ko, :],
                                x_sb[:, ko, off:off + sz],
                                start=(ko == 0), stop=(ko == 7))
                        nc.scalar.activation(
                            h1_sb[:, mo, off:off + sz], ps[:, :sz],
                            mybir.ActivationFunctionType.Relu,
                            bias=b1t[:, mo:mo + 1])

                # L2
                for mo in range(16):
                    w2t = wpool.tile([128, 32, 128], mybir.dt.bfloat16, tag="w2")
                    nc.sync.dma_start(w2t[:], d_w2[s].ap()[:, mo, :, :])
                    for off, sz in blks:
                        ps = psum.tile([128, 512], mybir.dt.float32, tag="ps")
                        for ko in range(32):
                            nc.tensor.matmul(
                                ps[:, :sz], w2t[:, ko, :],
                                h1_sb[:, ko, off:off + sz],
                                start=(ko == 0), stop=(ko == 31))
                        nc.scalar.activation(
                            h2_sb[:, mo, off:off + sz], ps[:, :sz],
                            mybir.ActivationFunctionType.Relu,
                            bias=b2t[:, mo:mo + 1])

                # L3 (no relu)
                for mo in range(8):
                    w3t = wpool.tile([128, 16, 128], mybir.dt.bfloat16, tag="w3")
                    nc.sync.dma_start(w3t[:], d_w3[s].ap()[:, mo, :, :])
                    for off, sz in blks:
                        ps = psum.tile([128, 512], mybir.dt.float32, tag="ps")
                        for ko in range(16):
                            nc.tensor.matmul(
                                ps[:, :sz], w3t[:, ko, :],
                                h2_sb[:, ko, off:off + sz],
                                start=(ko == 0), stop=(ko == 15))
                        ot = outp.tile([128, 512], mybir.dt.float32, tag="ot")
                        nc.scalar.activation(
                            ot[:, :sz], ps[:, :sz],
                            mybir.ActivationFunctionType.Identity,
                            bias=b3t[:, mo:mo + 1])
                        nc.sync.dma_start(d_out[s].ap()[mo, :, off:off + sz],
                                          ot[:, :sz])

    nc.compile()
    return nc


# ---------------------------------------------------------------- kernel

def _pack_weights(W1, b1, W2, b2, W3, b3):
    packed = []
    for e in range(E):
        w1p = np.ascontiguousarray(
            W1[e].reshape(8, 128, 32, 128).transpose(1, 2, 0, 3).astype(BF16))
        w2p = np.ascontiguousarray(
            W2[e].reshape(32, 128, 16, 128).transpose(1, 2, 0, 3).astype(BF16))
        w3p = np.ascontiguousarray(
            W3[e].reshape(16, 128, 8, 128).transpose(1, 2, 0, 3).astype(BF16))
        b1p = np.ascontiguousarray(b1[e].reshape(32, 128).T.astype(np.float32))
        b2p = np.ascontiguousarray(b2[e].reshape(16, 128).T.astype(np.float32))
        b3p = np.ascontiguousarray(b3[e].reshape(8, 128).T.astype(np.float32))
        packed.append((w1p, b1p, w2p, b2p, w3p, b3p))
    return packed


def kernel(x, W1, b1, W2, b2, W3, b3, Wg1, bg1, Wg2, bg2):
    x = np.asarray(x, np.float32)
    W1, b1 = np.asarray(W1, np.float32), np.asarray(b1, np.float32)
    W2, b2 = np.asarray(W2, np.float32), np.asarray(b2, np.float32)
    W3, b3 = np.asarray(W3, np.float32), np.asarray(b3, np.float32)
    Wg1, bg1 = np.asarray(Wg1, np.float32), np.asarray(bg1, np.float32)
    Wg2, bg2 = np.asarray(Wg2, np.float32), np.asarray(bg2, np.float32)

    gate_w, top_idx, top_w = _gate_and_route(x, Wg1, bg1, Wg2, bg2)

    # per-expert token lists (token b appears once per selected expert)
    tok_of_expert = [np.nonzero((top_idx == e).any(axis=1))[0] for e in range(E)]
    counts = np.array([len(t) for t in tok_of_expert])
    profile, assignment = plan_slots(counts)
    S = len(profile)

    key = profile
    if key not in _program_cache:
        _program_cache[key] = _build_program(profile)
    nc = _program_cache[key]

    packed = _pack_weights(W1, b1, W2, b2, W3, b3)
    xb = x.astype(BF16)

    in_maps = []
    core_slot_tokens = {}
    for core in range(NCORES):
        im = {}
        for s, C in enumerate(profile):
            e, off, ln = assignment.get((core, s), (0, 0, 0))
            toks = tok_of_expert[e][off:off + ln]
            core_slot_tokens[(core, s)] = (e, toks)
            xT = np.zeros((D, C), BF16)
            if ln:
                xT[:, :ln] = xb[toks].T
            im[f"xT{s}"] = np.ascontiguousarray(xT).reshape(8, 128, C)
            w1p, b1p, w2p, b2p, w3p, b3p = packed[e]
            im[f"w1_{s}"], im[f"b1_{s}"] = w1p, b1p
            im[f"w2_{s}"], im[f"b2_{s}"] = w2p, b2p
            im[f"w3_{s}"], im[f"b3_{s}"] = w3p, b3p
        in_maps.append(im)

    res = run_bass_kernel_spmd(nc, in_maps, core_ids=list(range(NCORES)),
                               trace=False)

    # combine: out[b] = sum_k top_w[b,k] * expert_out[top_idx[b,k]][b]
    out = np.zeros((B, DOUT), np.float32)
    wsel = np.zeros((B, E), np.float32)
    np.put_along_axis(wsel, top_idx, top_w, axis=1)
    for core in range(NCORES):
        for s in range(S):
            e, toks = core_slot_tokens[(core, s)]
            if len(toks) == 0:
                continue
            oT = res.results[core][f"outT{s}"].reshape(D, profile[s])
            out[toks] += wsel[toks, e][:, None] * oT[:, :len(toks)].T

    usage = gate_w.mean(axis=0)
    lbl = np.mean((usage - 1.0 / E) ** 2).astype(np.float32)
    return out, lbl


# revision 3
# speedup vs baseline: 83.5830x; 83.5830x over previous
"""MoE layer kernel for Trainium2 (8 NeuronCores, Bass/Tile).

Strategy: the reference runs all E=8 experts densely on all B=8192 tokens,
but only the top-2 experts per token contribute to the output. We compute
the (cheap) gate + top-2 routing on the host, then run only the routed
(token, expert) pairs on the device (4x less FLOPs than dense).

Device sharding: (expert, token-chunk) pieces are load-balanced across the
8 cores. Each core runs the same SPMD program with S weight "slots" of
static capacities (the profile is uniform across cores; computed at runtime
from the actual routing counts, then compiled in). Slot s of core c gets
one expert's weights and a chunk of that expert's tokens.

Math on device (per slot): h1 = relu(x@W1+b1); h2 = relu(h1@W2+b2);
out = h2@W3+b3, computed feature-major (features on partitions, tokens on
the free axis) in bf16 with fp32 PSUM accumulation.
"""

import numpy as np
import ml_dtypes

import concourse.bacc as bacc
import concourse.mybir as mybir
from concourse import tile
from concourse.bass_utils import run_bass_kernel_spmd

BF16 = ml_dtypes.bfloat16

D, H, H2, DOUT, E, B, TOP_K = 1024, 4096, 2048, 1024, 8, 8192, 2
NCORES = 8
CAP_MAX = 1280  # max slot capacity s.t. activations stay resident in SBUF
QUANT = 128

_program_cache: dict = {}


# ---------------------------------------------------------------- routing

def _gate_and_route(x, Wg1, bg1, Wg2, bg2):
    g = np.maximum(x @ Wg1 + bg1, 0.0)
    logits = g @ Wg2 + bg2
    m = logits.max(axis=-1, keepdims=True)
    e = np.exp(logits - m)
    gate_w = e / e.sum(axis=-1, keepdims=True)  # [B, E]

    idx = np.argpartition(-gate_w, TOP_K - 1, axis=1)[:, :TOP_K]
    vals = np.take_along_axis(gate_w, idx, 1)
    order = np.argsort(-vals, axis=1, kind="stable")
    top_idx = np.take_along_axis(idx, order, 1)  # [B, 2] descending
    top_v = np.take_along_axis(vals, order, 1)
    # renormalize with softmax over the selected gate *values*
    tm = top_v.max(axis=-1, keepdims=True)
    te = np.exp(top_v - tm)
    top_w = te / te.sum(axis=-1, keepdims=True)  # [B, 2]
    return gate_w, top_idx, top_w


# ------------------------------------------------------- slot-profile plan

def _feasible(profile, demands):
    """Greedy cover of per-expert demands with 8 instances of each slot size.

    Returns per-expert list of slot sizes used, or None."""
    pool = {}
    for c in profile:
        pool[c] = pool.get(c, 0) + NCORES
    sizes = sorted(pool, reverse=True)
    take = {e: [] for e in range(len(demands))}
    for e in np.argsort(-np.asarray(demands)):
        rem = demands[e]
        while rem > 0:
            avail = [c for c in sizes if pool[c] > 0]
            if not avail:
                return None
            if rem >= avail[0]:
                c = avail[0]
            else:
                c = min(c for c in avail if c >= rem)
            pool[c] -= 1
            take[e].append(c)
            rem -= c
    return take


def plan_slots(counts):
    """Pick a slot-capacity profile (uniform across cores) and assign
    (expert, piece) -> (core, slot)."""
    demands = [int(c) for c in counts]
    total = sum(demands)
    best = None
    for c1 in range(CAP_MAX, 0, -QUANT):
        for c2 in range(c1, -1, -QUANT):
            for c3 in range(c2, -1, -QUANT):
                for c4 in [0] if c3 == 0 else range(c3, -1, -QUANT):
                    prof = tuple(c for c in (c1, c2, c3, c4) if c > 0)
                    if NCORES * sum(prof) < total:
                        continue
                    cost = NCORES * sum(prof) + 500 * len(prof)
                    if best is not None and cost >= best[0]:
                        continue
                    take = _feasible(prof, demands)
                    if take is not None:
                        best = (cost, prof, take)
    assert best is not None, "no feasible slot profile"
    _, prof, take = best

    # map instances to (core, slot_pos): slot position s has 8 instances
    # (cores 0..7); duplicate sizes in prof occupy distinct positions.
    pos_of_size = {}
    for s, c in enumerate(prof):
        pos_of_size.setdefault(c, []).append(s)
    free = {c: [(core, s) for s in pos_of_size[c] for core in range(NCORES)]
            for c in pos_of_size}
    # assignment[(core, slot)] = (expert, tok_offset_in_expert_list, length)
    assignment = {}
    for e, sizes_taken in take.items():
        off = 0
        rem = int(counts[e])
        for c in sorted(sizes_taken, reverse=True):
            core, s = free[c].pop()
            ln = min(rem, c)
            assignment[(core, s)] = (e, off, ln)
            off += ln
            rem -= ln
    return prof, assignment


# ------------------------------------------------------- device program

def _build_program(profile, reps=1):
    nc = bacc.Bacc("TRN2", target_bir_lowering=False, debug=False)
    S = len(profile)
    d_in, d_w1, d_b1, d_w2, d_b2, d_w3, d_b3, d_out = [], [], [], [], [], [], [], []
    for s, C in enumerate(profile):
        d_in.append(nc.dram_tensor(f"xT{s}", (8, 128, C), mybir.dt.bfloat16,
                                   kind="ExternalInput"))
        d_w1.append(nc.dram_tensor(f"w1_{s}", (128, 32, 8, 128), mybir.dt.bfloat16,
                                   kind="ExternalInput"))
        d_b1.append(nc.dram_tensor(f"b1_{s}", (128, 32), mybir.dt.float32,
                                   kind="ExternalInput"))
        d_w2.append(nc.dram_tensor(f"w2_{s}", (128, 16, 32, 128), mybir.dt.bfloat16,
                                   kind="ExternalInput"))
        d_b2.append(nc.dram_tensor(f"b2_{s}", (128, 16), mybir.dt.float32,
                                   kind="ExternalInput"))
        d_w3.append(nc.dram_tensor(f"w3_{s}", (128, 8, 16, 128), mybir.dt.bfloat16,
                                   kind="ExternalInput"))
        d_b3.append(nc.dram_tensor(f"b3_{s}", (128, 8), mybir.dt.float32,
                                   kind="ExternalInput"))
        d_out.append(nc.dram_tensor(f"outT{s}", (8, 128, C), mybir.dt.float32,
                                    kind="ExternalOutput"))

    def nblocks(C):
        blocks, off = [], 0
        while off < C:
            sz = min(512, C - off)
            blocks.append((off, sz))
            off += sz
        return blocks

    import contextlib

    with tile.TileContext(nc) as tc:
        with (
            tc.tile_pool(name="acts", bufs=1) as acts,
            tc.tile_pool(name="wpool", bufs=3) as wpool,
            tc.tile_pool(name="bias", bufs=2) as bias,
            tc.tile_pool(name="outp", bufs=4) as outp,
            tc.tile_pool(name="psum", bufs=6, space="PSUM") as psum,
            tc.For_i(0, reps, 1) if reps > 1 else contextlib.nullcontext(),
        ):
            for s, C in enumerate(profile):
                blks = nblocks(C)
                x_sb = acts.tile([128, 8, C], mybir.dt.bfloat16, tag="x")
                h1_sb = acts.tile([128, 32, C], mybir.dt.bfloat16, tag="h1")
                h2_sb = acts.tile([128, 16, C], mybir.dt.bfloat16, tag="h2")
                b1t = bias.tile([128, 32], mybir.dt.float32, tag="b1")
                b2t = bias.tile([128, 16], mybir.dt.float32, tag="b2")
                b3t = bias.tile([128, 8], mybir.dt.float32, tag="b3")
                nc.sync.dma_start(b1t[:], d_b1[s].ap()[:])
                nc.sync.dma_start(b2t[:], d_b2[s].ap()[:])
                nc.sync.dma_start(b3t[:], d_b3[s].ap()[:])
                for ko in range(8):
                    nc.sync.dma_start(x_sb[:, ko, :], d_in[s].ap()[ko, :, :])

                # L1: h1[mo] = relu(sum_ko W1[ko,mo].T @ x[ko] + b1[mo])
                for mo in range(32):
                    w1t = wpool.tile([128, 8, 128], mybir.dt.bfloat16, tag="w1")
                    nc.sync.dma_start(w1t[:], d_w1[s].ap()[:, mo, :, :])
                    for off, sz in blks:
                        ps = psum.tile([128, 512], mybir.dt.float32, tag="ps")
                        for ko in range(8):
                            nc.tensor.matmul(
                                ps[:, :sz], w1t[:, ko, :],
                                x_sb[:, ko, off:off + sz],
                                start=(ko == 0), stop=(ko == 7))
                        nc.scalar.activation(
                            h1_sb[:, mo, off:off + sz], ps[:, :sz],
                            mybir.ActivationFunctionType.Relu,
                            bias=b1t[:, mo:mo + 1])

                # L2
                for mo in range(16):
                    w2t = wpool.tile([128, 32, 128], mybir.dt.bfloat16, tag="w2")
                    nc.sync.dma_start(w2t[:], d_w2[s].ap()[:, mo, :, :])
                    for off, sz in blks:
                        ps = psum.tile([128, 512], mybir.dt.float32, tag="ps")
                        for ko in range(32):
                            nc.tensor.matmul(
                                ps[:, :sz], w2t[:, ko, :],
                                h1_sb[:, ko, off:off + sz],
                                start=(ko == 0), stop=(ko == 31))
                        nc.scalar.activation(
                            h2_sb[:, mo, off:off + sz], ps[:, :sz],
                            mybir.ActivationFunctionType.Relu,
                            bias=b2t[:, mo:mo + 1])

                # L3 (no relu)
                for mo in range(8):
                    w3t = wpool.tile([128, 16, 128], mybir.dt.bfloat16, tag="w3")
                    nc.sync.dma_start(w3t[:], d_w3[s].ap()[:, mo, :, :])
                    for off, sz in blks:
                        ps = psum.tile([128, 512], mybir.dt.float32, tag="ps")
                        for ko in range(16):
                            nc.tensor.matmul(
                                ps[:, :sz], w3t[:, ko, :],
                                h2_sb[:, ko, off:off + sz],
                                start=(ko == 0), stop=(ko == 15))
                        ot = outp.tile([128, 512], mybir.dt.float32, tag="ot")
                        nc.scalar.activation(
                            ot[:, :sz], ps[:, :sz],
                            mybir.ActivationFunctionType.Identity,
                            bias=b3t[:, mo:mo + 1])
                        nc.sync.dma_start(d_out[s].ap()[mo, :, off:off + sz],
                                          ot[:, :sz])

    nc.compile()
    return nc


# ---------------------------------------------------------------- kernel

def _pack_weights(W1, b1, W2, b2, W3, b3):
    packed = []
    for e in range(E):
        w1p = np.ascontiguousarray(
            W1[e].reshape(8, 128, 32, 128).transpose(1, 2, 0, 3).astype(BF16))
        w2p = np.ascontiguousarray(
            W2[e].reshape(32, 128, 16, 128).transpose(1, 2, 0, 3).astype(BF16))
        w3p = np.ascontiguousarray(
            W3[e].reshape(16, 128, 8, 128).transpose(1, 2, 0, 3).astype(BF16))
        b1p = np.ascontiguousarray(b1[e].reshape(32, 128).T.astype(np.float32))
        b2p = np.ascontiguousarray(b2[e].reshape(16, 128).T.astype(np.float32))
        b3p = np.ascontiguousarray(b3[e].reshape(8, 128).T.astype(np.float32))
        packed.append((w1p, b1p, w2p, b2p, w3p, b3p))
    return packed


def kernel(x, W1, b1, W2, b2, W3, b3, Wg1, bg1, Wg2, bg2):
    x = np.asarray(x, np.float32)
    W1, b1 = np.asarray(W1, np.float32), np.asarray(b1, np.float32)
    W2, b2 = np.asarray(W2, np.float32), np.asarray(b2, np.float32)
    W3, b3 = np.asarray(W3, np.float32), np.asarray(b3, np.float32)
    Wg1, bg1 = np.asarray(Wg1, np.float32), np.asarray(bg1, np.float32)
    Wg2, bg2 = np.asarray(Wg2, np.float32), np.asarray(bg2, np.float32)

    gate_w, top_idx, top_w = _gate_and_route(x, Wg1, bg1, Wg2, bg2)

    # per-expert token lists (token b appears once per selected expert)
    tok_of_expert = [np.nonzero((top_idx == e).any(axis=1))[0] for e in range(E)]
    counts = np.array([len(t) for t in tok_of_expert])
    profile, assignment = plan_slots(counts)
    S = len(profile)

    key = profile
    if key not in _program_cache:
        _program_cache[key] = _build_program(profile)
    nc = _program_cache[key]

    packed = _pack_weights(W1, b1, W2, b2, W3, b3)
    xb = x.astype(BF16)

    in_maps = []
    core_slot_tokens = {}
    for core in range(NCORES):
        im = {}
        for s, C in enumerate(profile):
            e, off, ln = assignment.get((core, s), (0, 0, 0))
            toks = tok_of_expert[e][off:off + ln]
            core_slot_tokens[(core, s)] = (e, toks)
            xT = np.zeros((D, C), BF16)
            if ln:
                xT[:, :ln] = xb[toks].T
            im[f"xT{s}"] = np.ascontiguousarray(xT).reshape(8, 128, C)
            w1p, b1p, w2p, b2p, w3p, b3p = packed[e]
            im[f"w1_{s}"], im[f"b1_{s}"] = w1p, b1p
            im[f"w2_{s}"], im[f"b2_{s}"] = w2p, b2p
            im[f"w3_{s}"], im[f"b3_{s}"] = w3p, b3p
        in_maps.append(im)

    res = run_bass_kernel_spmd(nc, in_maps, core_ids=list(range(NCORES)),
                               trace=False)

    # combine: out[b] = sum_k top_w[b,k] * expert_out[top_idx[b,k]][b]
    out = np.zeros((B, DOUT), np.float32)
    wsel = np.zeros((B, E), np.float32)
    np.put_along_axis(wsel, top_idx, top_w, axis=1)
    for core in range(NCORES):
        for s in range(S):
            e, toks = core_slot_tokens[(core, s)]
            if len(toks) == 0:
                continue
            oT = res.results[core][f"outT{s}"].reshape(D, profile[s])
            out[toks] += wsel[toks, e][:, None] * oT[:, :len(toks)].T

    usage = gate_w.mean(axis=0)
    lbl = np.mean((usage - 1.0 / E) ** 2).astype(np.float32)
    return out, lbl


# revision 8
# speedup vs baseline: 88.3773x; 1.0574x over previous
"""MoE layer kernel for Trainium2 (8 NeuronCores, Bass/Tile).

Strategy: the reference runs all E=8 experts densely on all B=8192 tokens,
but only the top-2 experts per token contribute to the output. We compute
the (cheap) gate + top-2 routing on the host, then run only the routed
(token, expert) pairs on the device (4x less FLOPs than dense).

Device sharding: (expert, token-chunk) pieces are load-balanced across the
8 cores. Each core runs the same SPMD program with S weight "slots" of
static capacities (the profile is uniform across cores; computed at runtime
from the actual routing counts, then compiled in). Slot s of core c gets
one expert's weights and a chunk of that expert's tokens.

Math on device (per slot): h1 = relu(x@W1+b1); h2 = relu(h1@W2+b2);
out = h2@W3+b3, computed feature-major (features on partitions, tokens on
the free axis) in bf16 with fp32 PSUM accumulation.
"""

import numpy as np
import ml_dtypes

import concourse.bacc as bacc
import concourse.mybir as mybir
from concourse import tile
from concourse.bass_utils import run_bass_kernel_spmd

BF16 = ml_dtypes.bfloat16

D, H, H2, DOUT, E, B, TOP_K = 1024, 4096, 2048, 1024, 8, 8192, 2
NCORES = 8
CAP_MAX = 1280  # max slot capacity s.t. activations stay resident in SBUF
QUANT = 64

_program_cache: dict = {}


# ---------------------------------------------------------------- routing

def _gate_and_route(x, Wg1, bg1, Wg2, bg2):
    g = np.maximum(x @ Wg1 + bg1, 0.0)
    logits = g @ Wg2 + bg2
    m = logits.max(axis=-1, keepdims=True)
    e = np.exp(logits - m)
    gate_w = e / e.sum(axis=-1, keepdims=True)  # [B, E]

    idx = np.argpartition(-gate_w, TOP_K - 1, axis=1)[:, :TOP_K]
    vals = np.take_along_axis(gate_w, idx, 1)
    order = np.argsort(-vals, axis=1, kind="stable")
    top_idx = np.take_along_axis(idx, order, 1)  # [B, 2] descending
    top_v = np.take_along_axis(vals, order, 1)
    # renormalize with softmax over the selected gate *values*
    tm = top_v.max(axis=-1, keepdims=True)
    te = np.exp(top_v - tm)
    top_w = te / te.sum(axis=-1, keepdims=True)  # [B, 2]
    return gate_w, top_idx, top_w


# ------------------------------------------------------- slot-profile plan

def _feasible(profile, demands):
    """Greedy cover of per-expert demands with 8 instances of each slot size.

    Returns per-expert list of slot sizes used, or None."""
    pool = {}
    for c in profile:
        pool[c] = pool.get(c, 0) + NCORES
    sizes = sorted(pool, reverse=True)
    take = {e: [] for e in range(len(demands))}
    for e in np.argsort(-np.asarray(demands)):
        rem = demands[e]
        while rem > 0:
            avail = [c for c in sizes if pool[c] > 0]
            if not avail:
                return None
            if rem >= avail[0]:
                c = avail[0]
            else:
                c = min(c for c in avail if c >= rem)
            pool[c] -= 1
            take[e].append(c)
            rem -= c
    return take


def plan_slots(counts):
    """Pick a slot-capacity profile (uniform across cores) and assign
    (expert, piece) -> (core, slot)."""
    demands = [int(c) for c in counts]
    total = sum(demands)
    best = None
    for c1 in range(CAP_MAX, 0, -QUANT):
        for c2 in range(c1, -1, -QUANT):
            for c3 in range(c2, -1, -QUANT):
                for c4 in [0] if c3 == 0 else range(c3, -1, -QUANT):
                    prof = tuple(c for c in (c1, c2, c3, c4) if c > 0)
                    if NCORES * sum(prof) < total:
                        continue
                    # cost in token-equivalents: a slot's weight streaming
                    # (28MB bf16 ~ 78us ~ 210 tokens of PE time) puts a floor
                    # under small slots; plus fixed per-slot overhead.
                    cost = NCORES * sum(max(c, 208) for c in prof) \
                        + 500 * len(prof)
                    if best is not None and cost >= best[0]:
                        continue
                    take = _feasible(prof, demands)
                    if take is not None:
                        best = (cost, prof, take)
    assert best is not None, "no feasible slot profile"
    _, prof, take = best

    # map instances to (core, slot_pos): slot position s has 8 instances
    # (cores 0..7); duplicate sizes in prof occupy distinct positions.
    pos_of_size = {}
    for s, c in enumerate(prof):
        pos_of_size.setdefault(c, []).append(s)
    free = {c: [(core, s) for s in pos_of_size[c] for core in range(NCORES)]
            for c in pos_of_size}
    # assignment[(core, slot)] = (expert, tok_offset_in_expert_list, length)
    assignment = {}
    for e, sizes_taken in take.items():
        off = 0
        rem = int(counts[e])
        for c in sorted(sizes_taken, reverse=True):
            core, s = free[c].pop()
            ln = min(rem, c)
            assignment[(core, s)] = (e, off, ln)
            off += ln
            rem -= ln
    return prof, assignment


# ------------------------------------------------------- device program

def _build_program(profile, reps=1):
    nc = bacc.Bacc("TRN2", target_bir_lowering=False, debug=False)
    S = len(profile)
    d_in, d_w1, d_b1, d_w2, d_b2, d_w3, d_b3, d_out = [], [], [], [], [], [], [], []
    for s, C in enumerate(profile):
        d_in.append(nc.dram_tensor(f"xT{s}", (8, 128, C), mybir.dt.bfloat16,
                                   kind="ExternalInput"))
        d_w1.append(nc.dram_tensor(f"w1_{s}", (128, 32, 8, 128), mybir.dt.bfloat16,
                                   kind="ExternalInput"))
        d_b1.append(nc.dram_tensor(f"b1_{s}", (128, 32), mybir.dt.float32,
                                   kind="ExternalInput"))
        d_w2.append(nc.dram_tensor(f"w2_{s}", (128, 16, 32, 128), mybir.dt.bfloat16,
                                   kind="ExternalInput"))
        d_b2.append(nc.dram_tensor(f"b2_{s}", (128, 16), mybir.dt.float32,
                                   kind="ExternalInput"))
        d_w3.append(nc.dram_tensor(f"w3_{s}", (128, 8, 16, 128), mybir.dt.bfloat16,
                                   kind="ExternalInput"))
        d_b3.append(nc.dram_tensor(f"b3_{s}", (128, 8), mybir.dt.float32,
                                   kind="ExternalInput"))
        d_out.append(nc.dram_tensor(f"outT{s}", (8, 128, C), mybir.dt.float32,
                                    kind="ExternalOutput"))

    def nblocks(C):
        # split C into near-equal blocks of <=512 (balanced: avoids tiny
        # trailing matmuls whose fixed issue overhead is proportionally high)
        n = -(-C // 512)
        base, extra = divmod(C, n)
        blocks, off = [], 0
        for i in range(n):
            sz = base + (1 if i < extra else 0)
            blocks.append((off, sz))
            off += sz
        return blocks

    import contextlib

    with tile.TileContext(nc) as tc:
        with (
            tc.tile_pool(name="acts", bufs=1) as acts,
            tc.tile_pool(name="wpool", bufs=3) as wpool,
            tc.tile_pool(name="bias", bufs=2) as bias,
            tc.tile_pool(name="outp", bufs=4) as outp,
            tc.tile_pool(name="psum", bufs=6, space="PSUM") as psum,
            tc.For_i(0, reps, 1) if reps > 1 else contextlib.nullcontext(),
        ):
            for s, C in enumerate(profile):
                blks = nblocks(C)
                x_sb = acts.tile([128, 8, C], mybir.dt.bfloat16, tag="x")
                h1_sb = acts.tile([128, 32, C], mybir.dt.bfloat16, tag="h1")
                h2_sb = acts.tile([128, 16, C], mybir.dt.bfloat16, tag="h2")
                b1t = bias.tile([128, 32], mybir.dt.float32, tag="b1")
                b2t = bias.tile([128, 16], mybir.dt.float32, tag="b2")
                b3t = bias.tile([128, 8], mybir.dt.float32, tag="b3")
                nc.sync.dma_start(b1t[:], d_b1[s].ap()[:])
                nc.sync.dma_start(b2t[:], d_b2[s].ap()[:])
                nc.sync.dma_start(b3t[:], d_b3[s].ap()[:])
                for ko in range(8):
                    nc.sync.dma_start(x_sb[:, ko, :], d_in[s].ap()[ko, :, :])

                # Layers: out_feature tile (mo) outer; contraction (ko) next;
                # token block (nb) innermost so consecutive matmuls reuse the
                # stationary weight tile (amortizes LDWEIGHTS).
                def layer(nko, nmo, wt_pool_tag, wt_dram, src, dst_fn, act, bt):
                    for mo in range(nmo):
                        wt = wpool.tile([128, nko, 128], mybir.dt.bfloat16,
                                        tag=wt_pool_tag)
                        nc.sync.dma_start(wt[:], wt_dram.ap()[:, mo, :, :])
                        pss = []
                        for _bi in range(len(blks)):
                            ps = psum.tile([128, 512], mybir.dt.float32,
                                           tag="ps")
                            pss.append(ps)
                        for ko in range(nko):
                            for bi, (off, sz) in enumerate(blks):
                                nc.tensor.matmul(
                                    pss[bi][:, :sz], wt[:, ko, :],
                                    src(ko, off, sz),
                                    start=(ko == 0), stop=(ko == nko - 1))
                        for bi, (off, sz) in enumerate(blks):
                            dst_fn(mo, off, sz, pss[bi], act, bt)

                def store_act(h_sb):
                    def f(mo, off, sz, ps, act, bt):
                        nc.scalar.activation(
                            h_sb[:, mo, off:off + sz], ps[:, :sz], act,
                            bias=bt[:, mo:mo + 1])
                    return f

                def store_out(mo, off, sz, ps, act, bt):
                    ot = outp.tile([128, 512], mybir.dt.float32, tag="ot")
                    nc.scalar.activation(
                        ot[:, :sz], ps[:, :sz], act, bias=bt[:, mo:mo + 1])
                    nc.sync.dma_start(d_out[s].ap()[mo, :, off:off + sz],
                                      ot[:, :sz])

                relu = mybir.ActivationFunctionType.Relu
                ident = mybir.ActivationFunctionType.Identity
                layer(8, 32, "w1", d_w1[s],
                      lambda ko, off, sz: x_sb[:, ko, off:off + sz],
                      store_act(h1_sb), relu, b1t)
                layer(32, 16, "w2", d_w2[s],
                      lambda ko, off, sz: h1_sb[:, ko, off:off + sz],
                      store_act(h2_sb), relu, b2t)
                layer(16, 8, "w3", d_w3[s],
                      lambda ko, off, sz: h2_sb[:, ko, off:off + sz],
                      store_out, ident, b3t)

    nc.compile()
    return nc


# ---------------------------------------------------------------- kernel

def _pack_weights(W1, b1, W2, b2, W3, b3):
    packed = []
    for e in range(E):
        w1p = np.ascontiguousarray(
            W1[e].reshape(8, 128, 32, 128).transpose(1, 2, 0, 3).astype(BF16))
        w2p = np.ascontiguousarray(
            W2[e].reshape(32, 128, 16, 128).transpose(1, 2, 0, 3).astype(BF16))
        w3p = np.ascontiguousarray(
            W3[e].reshape(16, 128, 8, 128).transpose(1, 2, 0, 3).astype(BF16))
        b1p = np.ascontiguousarray(b1[e].reshape(32, 128).T.astype(np.float32))
        b2p = np.ascontiguousarray(b2[e].reshape(16, 128).T.astype(np.float32))
        b3p = np.ascontiguousarray(b3[e].reshape(8, 128).T.astype(np.float32))
        packed.append((w1p, b1p, w2p, b2p, w3p, b3p))
    return packed


def kernel(x, W1, b1, W2, b2, W3, b3, Wg1, bg1, Wg2, bg2):
    x = np.asarray(x, np.float32)
    W1, b1 = np.asarray(W1, np.float32), np.asarray(b1, np.float32)
    W2, b2 = np.asarray(W2, np.float32), np.asarray(b2, np.float32)
    W3, b3 = np.asarray(W3, np.float32), np.asarray(b3, np.float32)
    Wg1, bg1 = np.asarray(Wg1, np.float32), np.asarray(bg1, np.float32)
    Wg2, bg2 = np.asarray(Wg2, np.float32), np.asarray(bg2, np.float32)

    gate_w, top_idx, top_w = _gate_and_route(x, Wg1, bg1, Wg2, bg2)

    # per-expert token lists (token b appears once per selected expert)
    tok_of_expert = [np.nonzero((top_idx == e).any(axis=1))[0] for e in range(E)]
    counts = np.array([len(t) for t in tok_of_expert])
    profile, assignment = plan_slots(counts)
    S = len(profile)

    key = profile
    if key not in _program_cache:
        _program_cache[key] = _build_program(profile)
    nc = _program_cache[key]

    packed = _pack_weights(W1, b1, W2, b2, W3, b3)
    xb = x.astype(BF16)

    in_maps = []
    core_slot_tokens = {}
    for core in range(NCORES):
        im = {}
        for s, C in enumerate(profile):
            e, off, ln = assignment.get((core, s), (0, 0, 0))
            toks = tok_of_expert[e][off:off + ln]
            core_slot_tokens[(core, s)] = (e, toks)
            xT = np.zeros((D, C), BF16)
            if ln:
                xT[:, :ln] = xb[toks].T
            im[f"xT{s}"] = np.ascontiguousarray(xT).reshape(8, 128, C)
            w1p, b1p, w2p, b2p, w3p, b3p = packed[e]
            im[f"w1_{s}"], im[f"b1_{s}"] = w1p, b1p
            im[f"w2_{s}"], im[f"b2_{s}"] = w2p, b2p
            im[f"w3_{s}"], im[f"b3_{s}"] = w3p, b3p
        in_maps.append(im)

    res = run_bass_kernel_spmd(nc, in_maps, core_ids=list(range(NCORES)),
                               trace=False)

    # combine: out[b] = sum_k top_w[b,k] * expert_out[top_idx[b,k]][b]
    out = np.zeros((B, DOUT), np.float32)
    wsel = np.zeros((B, E), np.float32)
    np.put_along_axis(wsel, top_idx, top_w, axis=1)
    for core in range(NCORES):
        for s in range(S):
            e, toks = core_slot_tokens[(core, s)]
            if len(toks) == 0:
                continue
            oT = res.results[core][f"outT{s}"].reshape(D, profile[s])
            out[toks] += wsel[toks, e][:, None] * oT[:, :len(toks)].T

    usage = gate_w.mean(axis=0)
    lbl = np.mean((usage - 1.0 / E) ** 2).astype(np.float32)
    return out, lbl
